# revision 6
# baseline (speedup 1.0000x reference)
"""L2 self-attention (dist2 = q2+k2-2qk, attn=exp(-dist2*scale) row-normalized)
for Trainium2, 8 NeuronCores.

Sharding: B=4 batches x 2 core-groups; core c handles batch b=c//2 and heads
[3*(c%2), 3*(c%2)+3).  Each core computes its 3 heads' full 2048x2048 attention
(written directly to its slice of `attn`) plus the partial output projection
for its head block; the two half-head partials per batch are summed on host.

Math notes (exactness vs reference):
  attn[n,m] = exp(-(q2_n + k2_m - 2 S_nm)/8) / (sum_m ... + 1e-8)
The exp(-q2_n/8) factor cancels in the normalization (the 1e-8 makes that
inexact by ~1e-9 relative - far below fp32 noise), so we drop q2 entirely:
  attn[n,m] = exp(S_nm/4 - k2_m/8 - ln(den_n)),  den_n = sum_m exp(S/4 - k2/8)
The 1/4 is folded into wq on host.  k2_m/8 is folded into the matmul itself by
extending the contraction dim to 65: qT gets a row of ones, kT gets a row of
-k2/8.  ln(den) is applied as the per-partition ACT bias of the final exp, so
the attention tiles leave the scalar engine already normalized.

den_n is obtained for free from the A_T @ v pass: v gets a 65th column of
ones, so row 64 of the ho accumulator is exactly den.
"""

import sys

sys.path.insert(0, "/opt/trn_rl_repo")

from contextlib import ExitStack

import numpy as np

import concourse.bass as bass
import concourse.tile as tile
from concourse import bacc, mybir

F32 = mybir.dt.float32

B, N, D, H, HD = 4, 2048, 384, 6, 64
HPC = 3  # heads per core
NCORES = 8
KD = D // 128  # 3 k-slices of the D contraction


def _as_ap(t):
    return t if isinstance(t, bass.AP) else t.ap()


def l2attn_body(ctx: ExitStack, tc: tile.TileContext, outs, ins, n=N):
    nc = tc.nc
    xT, wqT, wkT, wvT, woT, bq, bk, bv = (
        _as_ap(ins["xT"]), _as_ap(ins["wqT"]), _as_ap(ins["wkT"]),
        _as_ap(ins["wvT"]), _as_ap(ins["woT"]),
        _as_ap(ins["bq"]), _as_ap(ins["bk"]), _as_ap(ins["bv"]),
    )
    attn3, pout = _as_ap(outs["attn3"]), _as_ap(outs["pout"])

    NT = n // 128          # 128-row tiles along n/m
    NH = max(n // 1024, 1)  # 1024-wide halves
    HW = min(n, 1024)      # half width
    NCK = HW // 512        # 512-chunks per half

    singles = ctx.enter_context(tc.tile_pool(name="singles", bufs=1))
    qk_pool = ctx.enter_context(tc.tile_pool(name="qk", bufs=2))
    tmp_pool = ctx.enter_context(tc.tile_pool(name="tmp", bufs=2))
    v_pool = ctx.enter_context(tc.tile_pool(name="v", bufs=2))
    at_pool = ctx.enter_context(tc.tile_pool(name="at", bufs=3))
    ao_pool = ctx.enter_context(tc.tile_pool(name="ao", bufs=4))
    sm_pool = ctx.enter_context(tc.tile_pool(name="sm", bufs=2))
    ps_st = ctx.enter_context(tc.tile_pool(name="ps_st", bufs=2, space="PSUM"))
    ps_ho = ctx.enter_context(tc.tile_pool(name="ps_ho", bufs=2, space="PSUM"))
    dram = ctx.enter_context(tc.tile_pool(name="dram", bufs=2, space="DRAM"))

    # ---- constants ----
    xT_sb = singles.tile([128, KD, n], F32)
    nc.sync.dma_start(out=xT_sb, in_=xT.rearrange("(a p) n -> p a n", p=128))
    wq_sb = singles.tile([128, KD, HPC * HD], F32)
    nc.sync.dma_start(out=wq_sb, in_=wqT.rearrange("(a p) j -> p a j", p=128))
    wk_sb = singles.tile([128, KD, HPC * HD], F32)
    nc.sync.dma_start(out=wk_sb, in_=wkT.rearrange("(a p) j -> p a j", p=128))
    wv_sb = singles.tile([128, KD, HPC * HD], F32)
    nc.sync.dma_start(out=wv_sb, in_=wvT.rearrange("(a p) j -> p a j", p=128))
    wo_sb = singles.tile([128, 2, D], F32)  # woT host-padded to [256, D]
    nc.sync.dma_start(out=wo_sb, in_=woT.rearrange("(a p) j -> p a j", p=128))
    bq_sb = singles.tile([HD, HPC], F32)
    nc.sync.dma_start(out=bq_sb, in_=bq.rearrange("(h d) -> d h", d=HD))
    bk_sb = singles.tile([HD, HPC], F32)
    nc.sync.dma_start(out=bk_sb, in_=bk.rearrange("(h d) -> d h", d=HD))
    bv_sb = singles.tile([128, HPC, HD], F32)
    nc.sync.dma_start(
        out=bv_sb, in_=bass.AP(tensor=bv.tensor, offset=bv.offset, ap=[[0, 128], [HD, HPC], [1, HD]])
    )
    ones_col = singles.tile([HD, 1], F32)
    nc.vector.memset(ones_col, 1.0)
    ones_row = singles.tile([1, HD], F32)
    nc.vector.memset(ones_row, 1.0)
    # normalized head outputs, transposed: rows = head-concat channel c
    hoT_a = singles.tile([128, n], F32)  # heads 0,1
    hoT_b = singles.tile([HD, n], F32)   # head 2

    for h in range(HPC):
        hs = h * HD
        # ---- P1: projections qT_ext/kT_ext [65, n]; v_ext [128, NT, 65] ----
        qT_ext = qk_pool.tile([65, n], F32, tag="qT")
        kT_ext = qk_pool.tile([65, n], F32, tag="kT")
        for c in range(n // 512):
            cs = bass.ts(c, 512)
            pq = ps_st.tile([HD, 512], F32, tag="st")
            for ks in range(KD):
                nc.tensor.matmul(
                    out=pq, lhsT=wq_sb[:, ks, hs:hs + HD], rhs=xT_sb[:, ks, cs],
                    start=(ks == 0), stop=(ks == KD - 1),
                )
            nc.scalar.activation(
                out=qT_ext[0:HD, cs], in_=pq,
                func=mybir.ActivationFunctionType.Identity, bias=bq_sb[:, h:h + 1],
            )
            pk = ps_st.tile([HD, 512], F32, tag="st")
            for ks in range(KD):
                nc.tensor.matmul(
                    out=pk, lhsT=wk_sb[:, ks, hs:hs + HD], rhs=xT_sb[:, ks, cs],
                    start=(ks == 0), stop=(ks == KD - 1),
                )
            nc.scalar.activation(
                out=kT_ext[0:HD, cs], in_=pk,
                func=mybir.ActivationFunctionType.Identity, bias=bk_sb[:, h:h + 1],
            )
        nc.vector.memset(qT_ext[HD:HD + 1, :], 1.0)
        # k2 row: ones-matmul over kT^2, scaled by -1/8 into row 64 of kT_ext
        ksq = tmp_pool.tile([HD, n], F32, tag="ksq")
        nc.vector.tensor_mul(ksq, kT_ext[0:HD, :], kT_ext[0:HD, :])
        for c in range(n // 512):
            cs = bass.ts(c, 512)
            pk2 = ps_st.tile([1, 512], F32, tag="st")
            nc.tensor.matmul(out=pk2, lhsT=ones_col, rhs=ksq[:, cs])
            nc.scalar.activation(
                out=kT_ext[HD:HD + 1, cs], in_=pk2,
                func=mybir.ActivationFunctionType.Copy, scale=-0.125,
            )
        v_ext = v_pool.tile([128, NT, HD + 1], F32)
        for mt in range(NT):
            pv = ps_st.tile([128, HD], F32, tag="st")
            for ks in range(KD):
                nc.tensor.matmul(
                    out=pv, lhsT=xT_sb[:, ks, bass.ts(mt, 128)],
                    rhs=wv_sb[:, ks, hs:hs + HD],
                    start=(ks == 0), stop=(ks == KD - 1),
                )
            nc.vector.tensor_add(v_ext[:, mt, 0:HD], pv, bv_sb[:, h, :])
        nc.vector.memset(v_ext[:, :, HD:HD + 1], 1.0)

        # ---- P2: A_T = exp(S_T'), ho^T = v_ext.T @ A_T  (row 64 = den) ----
        lnscr = dram.tile([1, n], F32)
        lncols = sm_pool.tile([128, NT], F32, tag="lncols")
        rb_sb = tmp_pool.tile([HD, n], F32, tag="rb")
        for half in range(NH):
            hw0 = half * HW
            ho_ps = ps_ho.tile([HD + 1, HW], F32, tag="ho")
            for mt in range(NT):
                stp = ps_st.tile([128, HW], F32, tag="st")
                for q in range(NCK):
                    nc.tensor.matmul(
                        out=stp[:, bass.ts(q, 512)],
                        lhsT=kT_ext[:, bass.ts(mt, 128)],
                        rhs=qT_ext[:, hw0 + q * 512:hw0 + (q + 1) * 512],
                        skip_group_check=True,
                    )
                at = at_pool.tile([128, HW], F32)
                nc.scalar.activation(
                    out=at, in_=stp, func=mybir.ActivationFunctionType.Exp
                )
                for q in range(NCK):
                    nc.tensor.matmul(
                        out=ho_ps[:, bass.ts(q, 512)], lhsT=v_ext[:, mt, :],
                        rhs=at[:, bass.ts(q, 512)],
                        start=(mt == 0), stop=(mt == NT - 1),
                        skip_group_check=True,
                    )
            # epilogue for this half: recip/ln of den, broadcast recip, norm ho
            recip = sm_pool.tile([1, HW], F32, tag="recip")
            nc.vector.reciprocal(out=recip, in_=ho_ps[HD:HD + 1, :])
            lnr = sm_pool.tile([1, HW], F32, tag="lnr")
            nc.scalar.activation(
                out=lnr, in_=recip, func=mybir.ActivationFunctionType.Ln
            )
            nc.sync.dma_start(out=lnscr[:, hw0:hw0 + HW], in_=lnr)
            for c in range(NCK):
                prb = ps_st.tile([HD, 512], F32, tag="st")
                nc.tensor.matmul(
                    out=prb, lhsT=ones_row, rhs=recip[:, bass.ts(c, 512)],
                    skip_group_check=True,
                )
                nc.vector.tensor_copy(rb_sb[:, hw0 + c * 512:hw0 + (c + 1) * 512], prb)
            dst = hoT_a[hs:hs + HD, :] if h < 2 else hoT_b
            nc.vector.tensor_mul(
                dst[:, hw0:hw0 + HW], ho_ps[0:HD, :], rb_sb[:, hw0:hw0 + HW]
            )
        nc.sync.dma_start(
            out=lncols, in_=lnscr[0, :].rearrange("(t p) -> p t", p=128)
        )

        # ---- P3: attn tiles [n,m] = exp(S' + lnbias), already normalized ----
        for nt in range(NT):
            for half in range(NH):
                hw0 = half * HW
                sp = ps_st.tile([128, HW], F32, tag="st")
                for q in range(NCK):
                    nc.tensor.matmul(
                        out=sp[:, bass.ts(q, 512)],
                        lhsT=qT_ext[:, bass.ts(nt, 128)],
                        rhs=kT_ext[:, hw0 + q * 512:hw0 + (q + 1) * 512],
                        skip_group_check=True,
                    )
                ob = ao_pool.tile([128, HW], F32)
                nc.scalar.activation(
                    out=ob, in_=sp, func=mybir.ActivationFunctionType.Exp,
                    bias=lncols[:, nt:nt + 1],
                )
                nc.sync.dma_start(
                    out=attn3[h, bass.ts(nt, 128), hw0:hw0 + HW], in_=ob
                )

    # ---- P4: partial output projection pout = hoT.T @ woT_block ----
    for nt in range(NT):
        pp = ps_st.tile([128, D], F32, tag="st")
        nc.tensor.matmul(
            out=pp, lhsT=hoT_a[:, bass.ts(nt, 128)], rhs=wo_sb[:, 0, :],
            start=True, stop=False, skip_group_check=True,
        )
        nc.tensor.matmul(
            out=pp, lhsT=hoT_b[:, bass.ts(nt, 128)], rhs=wo_sb[0:HD, 1, :],
            start=False, stop=True, skip_group_check=True,
        )
        po = sm_pool.tile([128, D], F32, tag="po")
        nc.scalar.copy(out=po, in_=pp)
        nc.sync.dma_start(out=pout[bass.ts(nt, 128), :], in_=po)


def build_program(n=N):
    nc = bacc.Bacc("TRN2", target_bir_lowering=False, debug=False)
    ins = {
        "xT": nc.dram_tensor("xT", [D, n], F32, kind="ExternalInput"),
        "wqT": nc.dram_tensor("wqT", [D, HPC * HD], F32, kind="ExternalInput"),
        "wkT": nc.dram_tensor("wkT", [D, HPC * HD], F32, kind="ExternalInput"),
        "wvT": nc.dram_tensor("wvT", [D, HPC * HD], F32, kind="ExternalInput"),
        "woT": nc.dram_tensor("woT", [256, D], F32, kind="ExternalInput"),
        "bq": nc.dram_tensor("bq", [HPC * HD], F32, kind="ExternalInput"),
        "bk": nc.dram_tensor("bk", [HPC * HD], F32, kind="ExternalInput"),
        "bv": nc.dram_tensor("bv", [HPC * HD], F32, kind="ExternalInput"),
    }
    outs = {
        "attn3": nc.dram_tensor("attn3", [HPC, n, n], F32, kind="ExternalOutput"),
        "pout": nc.dram_tensor("pout", [n, D], F32, kind="ExternalOutput"),
    }
    with ExitStack() as ctx:
        tc = ctx.enter_context(tile.TileContext(nc))
        l2attn_body(ctx, tc, outs, ins, n=n)
    nc.compile()
    return nc


def make_in_maps(x, wq, bq, wk, bk, wv, bv, wo, n=N):
    """Per-core input dict. Core c: batch c//2, head block c%2."""
    in_maps = []
    for c in range(NCORES):
        b, hb = c // 2, c % 2
        hs = hb * HPC * HD
        woT = np.zeros((256, D), np.float32)
        woT[0:HPC * HD] = wo.T[hs:hs + HPC * HD, :]
        in_maps.append({
            "xT": np.ascontiguousarray(x[b].T.astype(np.float32))[:, :n],
            "wqT": np.ascontiguousarray(wq.T[:, hs:hs + HPC * HD] * 0.25),
            "wkT": np.ascontiguousarray(wk.T[:, hs:hs + HPC * HD]),
            "wvT": np.ascontiguousarray(wv.T[:, hs:hs + HPC * HD]),
            "woT": woT,
            "bq": np.ascontiguousarray(bq[hs:hs + HPC * HD] * 0.25),
            "bk": np.ascontiguousarray(bk[hs:hs + HPC * HD]),
            "bv": np.ascontiguousarray(bv[hs:hs + HPC * HD]),
        })
    return in_maps


def gather(results, bo):
    attn = np.empty((B, H, N, N), np.float32)
    out = np.zeros((B, N, D), np.float32)
    for c in range(NCORES):
        b, hb = c // 2, c % 2
        attn[b, hb * HPC:(hb + 1) * HPC] = results[c]["attn3"]
        out[b] += results[c]["pout"]
    out += bo.astype(np.float32)
    return out, attn


_CACHE = {}


def kernel(**inputs):
    from concourse.bass_utils import run_bass_kernel_spmd

    args = {k: np.asarray(v, np.float32) for k, v in inputs.items()}
    if "nc" not in _CACHE:
        _CACHE["nc"] = build_program()
    in_maps = make_in_maps(
        args["x"], args["wq"], args["bq"], args["wk"], args["bk"],
        args["wv"], args["bv"], args["wo"],
    )
    res = run_bass_kernel_spmd(_CACHE["nc"], in_maps, list(range(NCORES))).results
    return gather(res, args["bo"])
